# revision 6
# baseline (speedup 1.0000x reference)
"""MoE block (RMSNorm + top-4 router + 32-expert GLU FFN) on 8 TRN2 NeuronCores.

Expert-parallel: core c owns experts [4c, 4c+4). Each core redundantly
computes the (tiny) RMSNorm + router on all 32 experts, then runs a dense
masked FFN over all 64 tokens for its own 4 experts, scaling each expert's
contribution by the routing weight (0 for unrouted tokens). gate_w/gate_b
are passed to each core with its own 4 experts permuted to rows 0..3, so the
SPMD program always reads routing columns 0..3 — no core-id branching.

The host sums the 8 partial (T, D) outputs and adds the residual — that is
the "unshard" for expert parallelism.
"""

import sys
import types

sys.path.insert(0, "/opt/trn_rl_repo")

import numpy as np

D = 640
I = 640
E = 32
T = 64
K = 4
EPS = 1e-5
LIMIT = 7.0
BETA = 1.702
NCORES = 8
EPC = E // NCORES          # experts per core
NCH = D // 128             # 5 contraction chunks of 128
NIH = 2 * I // 128         # 10 partition chunks of h

TRACE = False
LAST_EXEC_NS = None

_NC = None


def _ensure_ntff_hook():
    """boot() skips NTFF hook registration (image antenv lacks axon_hooks);
    provide the module so bass_utils can profile when TRACE=True."""
    if "antenv.axon_hooks" in sys.modules:
        return
    try:
        from trn_agent_boot.trn_boot import _ntff_profile_via_ctypes
        hook = _ntff_profile_via_ctypes("/opt/axon/libaxon_pjrt.so")
    except Exception:
        hook = None
    m = types.ModuleType("antenv.axon_hooks")
    m.get_axon_ntff_profile_hook = lambda: hook
    m.set_axon_ntff_profile_hook = lambda h: None
    sys.modules["antenv.axon_hooks"] = m


def _build():
    import concourse.bass as bass
    import concourse.bacc as bacc
    import concourse.tile as tile
    from concourse import mybir
    from concourse.masks import make_identity

    f32 = mybir.dt.float32
    AF = mybir.ActivationFunctionType
    OP = mybir.AluOpType

    nc = bacc.Bacc("TRN2", target_bir_lowering=False, debug=False,
                   num_devices=NCORES)
    dx = nc.dram_tensor("x", (D, T), f32, kind="ExternalInput")
    dnw = nc.dram_tensor("norm_w", (D,), f32, kind="ExternalInput")
    dgw = nc.dram_tensor("gate_w", (E, D), f32, kind="ExternalInput")
    dgb = nc.dram_tensor("gate_b", (E,), f32, kind="ExternalInput")
    dw1 = nc.dram_tensor("w1", (EPC, D, 2 * I), f32, kind="ExternalInput")
    db1 = nc.dram_tensor("b1", (EPC, 2 * I), f32, kind="ExternalInput")
    dw2 = nc.dram_tensor("w2", (EPC, I, D), f32, kind="ExternalInput")
    db2 = nc.dram_tensor("b2", (EPC, D), f32, kind="ExternalInput")
    dout = nc.dram_tensor("out", (T, D), f32, kind="ExternalOutput")

    with tile.TileContext(nc) as tc:
        with (
            tc.tile_pool(name="consts", bufs=1) as consts,
            tc.tile_pool(name="small", bufs=2) as small,
            tc.tile_pool(name="wpool", bufs=3) as wpool,
            tc.tile_pool(name="hpool", bufs=3) as hpool,
            tc.tile_pool(name="ps_misc", bufs=1, space="PSUM") as ps_misc,
            tc.tile_pool(name="ps_h", bufs=4, space="PSUM") as ps_h,
            tc.tile_pool(name="ps_o", bufs=1, space="PSUM") as ps_o,
        ):
            # ---- expert weight streams (big; SP HWDGE ring) ----
            w1r = dw1.ap().rearrange("e (c p) i -> e p c i", p=128)
            w2r = dw2.ap().rearrange("e (c p) i -> e p c i", p=128)
            w1_tiles, w2_tiles = [], []
            for e in range(EPC):
                w1_t = wpool.tile([128, NCH, 2 * I], f32, tag="w1")
                nc.sync.dma_start(out=w1_t, in_=w1r[e])
                w2_t = wpool.tile([128, NCH, D], f32, tag="w2")
                nc.sync.dma_start(out=w2_t, in_=w2r[e])
                w1_tiles.append(w1_t)
                w2_tiles.append(w2_t)

            # ---- small input loads (gpsimd SWDGE ring, parallel to above) --
            x_t = consts.tile([128, NCH, T], f32)
            nc.gpsimd.dma_start(out=x_t,
                                in_=dx.ap().rearrange("(c p) t -> p c t", p=128))
            nw_t = consts.tile([128, NCH], f32)
            nc.gpsimd.dma_start(out=nw_t,
                                in_=dnw.ap().rearrange("(c p) -> p c", p=128))
            gwT = consts.tile([128, NCH, E], f32)
            gw_c = dgw.ap().rearrange("e (c p) -> c p e", p=128)
            for c in range(NCH):
                nc.gpsimd.dma_start(out=gwT[:, c, :], in_=gw_c[c])
            gb_b = consts.tile([T, E], f32)
            gb_base = dgb.ap()
            nc.gpsimd.dma_start(
                out=gb_b,
                in_=bass.AP(tensor=gb_base.tensor, offset=0,
                            ap=[[0, T], [1, E]]))
            b1T = consts.tile([128, NIH, EPC], f32)
            b1_c = db1.ap().rearrange("e (c p) -> c p e", p=128)
            for c in range(NIH):
                nc.gpsimd.dma_start(out=b1T[:, c, :], in_=b1_c[c])
            b2_t = consts.tile([EPC, D], f32)
            nc.gpsimd.dma_start(out=b2_t, in_=db2.ap())
            ones128 = consts.tile([128, 128], f32)
            nc.vector.memset(ones128, 1.0)
            eps_t = consts.tile([128, 1], f32)
            nc.vector.memset(eps_t, EPS)
            id64 = consts.tile([T, T], f32)
            make_identity(nc, id64)

            # ---- RMSNorm (x is (D, T); D on partitions) ----
            xx = small.tile([128, NCH, T], f32, tag="xx")
            nc.vector.tensor_mul(xx, x_t, x_t)
            ps_ss = ps_misc.tile([128, T], f32, tag="misc")
            for c in range(NCH):
                # ones.T @ xx chunk: broadcasts sum over D to all partitions
                nc.tensor.matmul(ps_ss, ones128, xx[:, c, :],
                                 start=(c == 0), stop=(c == NCH - 1))
            sq = small.tile([128, T], f32, tag="sq")
            nc.scalar.activation(sq, ps_ss, AF.Sqrt, bias=eps_t, scale=1.0 / D)
            rstd = small.tile([128, T], f32, tag="rstd")
            nc.vector.reciprocal(rstd, sq)
            normed = consts.tile([128, NCH, T], f32)
            for c in range(NCH):
                xn = small.tile([128, T], f32, tag="xn")
                nc.vector.tensor_scalar_mul(xn, x_t[:, c, :], nw_t[:, c:c + 1])
                nc.vector.tensor_mul(normed[:, c, :], xn, rstd)

            # ---- router: gate, top-4, softmax, dense routing matrix A ----
            ps_g = ps_misc.tile([T, E], f32, tag="misc")
            for c in range(NCH):
                nc.tensor.matmul(ps_g, normed[:, c, :], gwT[:, c, :],
                                 start=(c == 0), stop=(c == NCH - 1))
            g_sb = small.tile([T, E], f32, tag="g")
            nc.vector.tensor_add(g_sb, ps_g, gb_b)
            m8 = small.tile([T, 8], f32, tag="m8")
            nc.vector.max(m8, g_sb)
            negm = small.tile([T, 1], f32, tag="negm")
            nc.scalar.mul(negm, m8[:, 0:1], -1.0)
            s4 = small.tile([T, K], f32, tag="s4")
            nc.scalar.activation(s4, m8[:, 0:K], AF.Exp, bias=negm, scale=1.0)
            den = small.tile([T, 1], f32, tag="den")
            nc.vector.reduce_sum(den, s4, axis=mybir.AxisListType.X)
            rden = small.tile([T, 1], f32, tag="rden")
            nc.vector.reciprocal(rden, den)
            ew = small.tile([T, K], f32, tag="ew")
            nc.vector.tensor_scalar_mul(ew, s4, rden)

            A = small.tile([T, E], f32, tag="A")
            for k in range(K):
                msk = small.tile([T, E], f32, tag="msk")
                nc.vector.tensor_scalar(msk, g_sb, m8[:, k:k + 1], None,
                                        op0=OP.is_equal)
                wm = small.tile([T, E], f32, tag="wm")
                nc.vector.tensor_scalar_mul(wm, msk, ew[:, k:k + 1])
                if k == 0:
                    nc.vector.tensor_copy(A, wm)
                else:
                    nc.vector.tensor_add(A, A, wm)
            # h_act is computed as silu(beta*glu)*(lin+1) = beta * true value;
            # fold the 1/beta into the per-expert routing scale.
            A_div = small.tile([T, E], f32, tag="A_div")
            nc.vector.tensor_scalar_mul(A_div, A, 1.0 / BETA)
            ps_a = ps_misc.tile([K, T], f32, tag="misc")
            nc.tensor.transpose(ps_a, A[:, 0:K], id64)
            a4t = small.tile([K, T], f32, tag="a4t")
            nc.scalar.copy(a4t, ps_a)

            # ---- bias-2 base: acc = A[:, :4] @ b2_shard ----
            acc = consts.tile([T, D], f32)
            ps_b1 = ps_o.tile([T, 512], f32, tag="o1", bufs=2)
            nc.tensor.matmul(ps_b1, a4t, b2_t[:, 0:512], start=True, stop=True)
            nc.scalar.copy(acc[:, 0:512], ps_b1)
            ps_b2 = ps_o.tile([T, 128], f32, tag="o2")
            nc.tensor.matmul(ps_b2, a4t, b2_t[:, 512:640], start=True, stop=True)
            nc.scalar.copy(acc[:, 512:640], ps_b2)

            # ---- experts: dense masked GLU FFN ----
            for e in range(EPC):
                w1_t, w2_t = w1_tiles[e], w2_tiles[e]
                ps_o1 = ps_o.tile([T, 512], f32, tag="o1", bufs=2)
                ps_o2 = ps_o.tile([T, 128], f32, tag="o2")
                for c in range(NCH):
                    ps_glu = ps_h.tile([128, T], f32, tag="h")
                    ps_lin = ps_h.tile([128, T], f32, tag="h")
                    for d in range(NCH):
                        nc.tensor.matmul(
                            ps_glu, w1_t[:, d, 128 * c:128 * (c + 1)],
                            normed[:, d, :],
                            start=(d == 0), stop=(d == NCH - 1))
                    for d in range(NCH):
                        nc.tensor.matmul(
                            ps_lin, w1_t[:, d, I + 128 * c:I + 128 * (c + 1)],
                            normed[:, d, :],
                            start=(d == 0), stop=(d == NCH - 1))
                    gm = hpool.tile([128, T], f32, tag="gm")
                    nc.vector.tensor_scalar(gm, ps_glu, b1T[:, c, e:e + 1],
                                            LIMIT, op0=OP.add, op1=OP.min)
                    p_ = hpool.tile([128, T], f32, tag="p")
                    nc.scalar.activation(p_, gm, AF.Silu, scale=BETA)
                    l1 = hpool.tile([128, T], f32, tag="l")
                    nc.vector.tensor_scalar(l1, ps_lin,
                                            b1T[:, NCH + c, e:e + 1],
                                            LIMIT, op0=OP.add, op1=OP.min)
                    nc.vector.tensor_scalar(l1, l1, -LIMIT, 1.0,
                                            op0=OP.max, op1=OP.add)
                    hact = hpool.tile([128, T], f32, tag="hact")
                    nc.vector.tensor_mul(hact, p_, l1)
                    nc.tensor.matmul(ps_o1, hact, w2_t[:, c, 0:512],
                                     start=(c == 0), stop=(c == NCH - 1))
                    nc.tensor.matmul(ps_o2, hact, w2_t[:, c, 512:640],
                                     start=(c == 0), stop=(c == NCH - 1))
                sc1 = small.tile([T, 512], f32, tag="sc1")
                nc.vector.tensor_scalar_mul(sc1, ps_o1, A_div[:, e:e + 1])
                nc.vector.tensor_add(acc[:, 0:512], acc[:, 0:512], sc1)
                sc2 = small.tile([T, 128], f32, tag="sc2")
                nc.vector.tensor_scalar_mul(sc2, ps_o2, A_div[:, e:e + 1])
                nc.vector.tensor_add(acc[:, 512:640], acc[:, 512:640], sc2)

            nc.sync.dma_start(out=dout.ap(), in_=acc)

    nc.finalize()
    return nc


def _get_nc():
    global _NC
    if _NC is None:
        _ensure_ntff_hook()
        _NC = _build()
    return _NC


def kernel(**inputs):
    global LAST_EXEC_NS
    nc = _get_nc()
    from concourse.bass_utils import run_bass_kernel_spmd

    x = np.ascontiguousarray(np.asarray(inputs["x"], dtype=np.float32))
    norm_w = np.ascontiguousarray(np.asarray(inputs["norm_w"], np.float32))
    gate_w = np.ascontiguousarray(np.asarray(inputs["gate_w"], np.float32))
    gate_b = np.ascontiguousarray(np.asarray(inputs["gate_b"], np.float32))
    w1 = np.asarray(inputs["w1"], np.float32)
    b1 = np.asarray(inputs["b1"], np.float32)
    w2 = np.asarray(inputs["w2"], np.float32)
    b2 = np.asarray(inputs["b2"], np.float32)

    x2 = np.ascontiguousarray(x[0, :, 0, :])  # (D, T)
    in_maps = []
    for c in range(NCORES):
        lo, hi = EPC * c, EPC * (c + 1)
        perm = np.r_[lo:hi, 0:lo, hi:E]
        in_maps.append({
            "x": x2,
            "norm_w": norm_w,
            "gate_w": np.ascontiguousarray(gate_w[perm]),
            "gate_b": np.ascontiguousarray(gate_b[perm]),
            "w1": np.ascontiguousarray(w1[lo:hi]),
            "b1": np.ascontiguousarray(b1[lo:hi]),
            "w2": np.ascontiguousarray(w2[lo:hi]),
            "b2": np.ascontiguousarray(b2[lo:hi]),
        })

    res = run_bass_kernel_spmd(nc, in_maps, core_ids=list(range(NCORES)),
                               trace=TRACE)
    LAST_EXEC_NS = res.exec_time_ns
    total = np.sum([r["out"] for r in res.results], axis=0)  # (T, D)
    return (x + total.T[None, :, None, :]).astype(np.float32)


# revision 8
# speedup vs baseline: 1.0047x; 1.0047x over previous
"""MoE block (RMSNorm + top-4 router + 32-expert GLU FFN) on 8 TRN2 NeuronCores.

Expert-parallel: core c owns experts [4c, 4c+4). Each core redundantly
computes the (tiny) RMSNorm + router on all 32 experts, then runs a dense
masked FFN over all 64 tokens for its own 4 experts, scaling each expert's
contribution by the routing weight (0 for unrouted tokens). gate_w/gate_b
are passed to each core with its own 4 experts permuted to rows 0..3, so the
SPMD program always reads routing columns 0..3 — no core-id branching.

The host sums the 8 partial (T, D) outputs and adds the residual — that is
the "unshard" for expert parallelism.
"""

import sys
import types

sys.path.insert(0, "/opt/trn_rl_repo")

import numpy as np

D = 640
I = 640
E = 32
T = 64
K = 4
EPS = 1e-5
LIMIT = 7.0
BETA = 1.702
NCORES = 8
EPC = E // NCORES          # experts per core
NCH = D // 128             # 5 contraction chunks of 128
NIH = 2 * I // 128         # 10 partition chunks of h

TRACE = False
PROF_DIR = None
LAST_EXEC_NS = None

_NC = None


def _ensure_ntff_hook():
    """boot() skips NTFF hook registration (image antenv lacks axon_hooks);
    provide the module so bass_utils can profile when TRACE=True."""
    if "antenv.axon_hooks" in sys.modules:
        return
    try:
        from trn_agent_boot.trn_boot import _ntff_profile_via_ctypes
        hook = _ntff_profile_via_ctypes("/opt/axon/libaxon_pjrt.so")
    except Exception:
        hook = None
    m = types.ModuleType("antenv.axon_hooks")
    m.get_axon_ntff_profile_hook = lambda: hook
    m.set_axon_ntff_profile_hook = lambda h: None
    sys.modules["antenv.axon_hooks"] = m


def _build():
    import concourse.bass as bass
    import concourse.bacc as bacc
    import concourse.tile as tile
    from concourse import mybir
    from concourse.masks import make_identity

    f32 = mybir.dt.float32
    AF = mybir.ActivationFunctionType
    OP = mybir.AluOpType

    nc = bacc.Bacc("TRN2", target_bir_lowering=False, debug=False,
                   num_devices=NCORES)
    dx = nc.dram_tensor("x", (D, T), f32, kind="ExternalInput")
    dnw = nc.dram_tensor("norm_w", (D,), f32, kind="ExternalInput")
    dgw = nc.dram_tensor("gate_w", (E, D), f32, kind="ExternalInput")
    dgb = nc.dram_tensor("gate_b", (E,), f32, kind="ExternalInput")
    dw1 = nc.dram_tensor("w1", (EPC, D, 2 * I), f32, kind="ExternalInput")
    db1 = nc.dram_tensor("b1", (EPC, 2 * I), f32, kind="ExternalInput")
    dw2 = nc.dram_tensor("w2", (EPC, I, D), f32, kind="ExternalInput")
    db2 = nc.dram_tensor("b2", (EPC, D), f32, kind="ExternalInput")
    dout = nc.dram_tensor("out", (T, D), f32, kind="ExternalOutput")

    with tile.TileContext(nc) as tc:
        with (
            tc.tile_pool(name="consts", bufs=1) as consts,
            tc.tile_pool(name="small", bufs=2) as small,
            tc.tile_pool(name="wpool", bufs=3) as wpool,
            tc.tile_pool(name="hpool", bufs=3) as hpool,
            tc.tile_pool(name="ps_misc", bufs=1, space="PSUM") as ps_misc,
            tc.tile_pool(name="ps_h", bufs=4, space="PSUM") as ps_h,
            tc.tile_pool(name="ps_o", bufs=1, space="PSUM") as ps_o,
        ):
            # ---- expert weight streams (big; SP HWDGE ring) ----
            w1r = dw1.ap().rearrange("e (c p) i -> e p c i", p=128)
            w2r = dw2.ap().rearrange("e (c p) i -> e p c i", p=128)
            w1_tiles, w2_tiles = [], []
            for e in range(EPC):
                w1_t = wpool.tile([128, NCH, 2 * I], f32, tag="w1")
                nc.sync.dma_start(out=w1_t, in_=w1r[e])
                w2_t = wpool.tile([128, NCH, D], f32, tag="w2")
                nc.sync.dma_start(out=w2_t, in_=w2r[e])
                w1_tiles.append(w1_t)
                w2_tiles.append(w2_t)

            # ---- small input loads (gpsimd SWDGE ring, parallel to above) --
            x_t = consts.tile([128, NCH, T], f32)
            nc.gpsimd.dma_start(out=x_t,
                                in_=dx.ap().rearrange("(c p) t -> p c t", p=128))
            nw_t = consts.tile([128, NCH], f32)
            nc.gpsimd.dma_start(out=nw_t,
                                in_=dnw.ap().rearrange("(c p) -> p c", p=128))
            gwT = consts.tile([128, NCH, E], f32)
            gw_c = dgw.ap().rearrange("e (c p) -> c p e", p=128)
            for c in range(NCH):
                nc.gpsimd.dma_start(out=gwT[:, c, :], in_=gw_c[c])
            gb_b = consts.tile([T, E], f32)
            gb_base = dgb.ap()
            nc.gpsimd.dma_start(
                out=gb_b,
                in_=bass.AP(tensor=gb_base.tensor, offset=0,
                            ap=[[0, T], [1, E]]))
            b1T = consts.tile([128, NIH, EPC], f32)
            b1_c = db1.ap().rearrange("e (c p) -> c p e", p=128)
            for c in range(NIH):
                nc.gpsimd.dma_start(out=b1T[:, c, :], in_=b1_c[c])
            b2_t = consts.tile([EPC, D], f32)
            nc.gpsimd.dma_start(out=b2_t, in_=db2.ap())
            ones128 = consts.tile([128, 128], f32)
            nc.vector.memset(ones128, 1.0)
            eps_t = consts.tile([128, 1], f32)
            nc.vector.memset(eps_t, EPS)
            id64 = consts.tile([T, T], f32)
            make_identity(nc, id64)

            # ---- RMSNorm (x is (D, T); D on partitions) ----
            xx = small.tile([128, NCH, T], f32, tag="xx")
            nc.vector.tensor_mul(xx, x_t, x_t)
            ps_ss = ps_misc.tile([128, T], f32, tag="misc")
            for c in range(NCH):
                # ones.T @ xx chunk: broadcasts sum over D to all partitions
                nc.tensor.matmul(ps_ss, ones128, xx[:, c, :],
                                 start=(c == 0), stop=(c == NCH - 1))
            sq = small.tile([128, T], f32, tag="sq")
            nc.scalar.activation(sq, ps_ss, AF.Sqrt, bias=eps_t, scale=1.0 / D)
            rstd = small.tile([128, T], f32, tag="rstd")
            nc.vector.reciprocal(rstd, sq)
            normed = consts.tile([128, NCH, T], f32)
            for c in range(NCH):
                xn = small.tile([128, T], f32, tag="xn")
                nc.vector.tensor_scalar_mul(xn, x_t[:, c, :], nw_t[:, c:c + 1])
                nc.vector.tensor_mul(normed[:, c, :], xn, rstd)

            # ---- router: gate, top-4, softmax, dense routing matrix A ----
            ps_g = ps_misc.tile([T, E], f32, tag="misc")
            for c in range(NCH):
                nc.tensor.matmul(ps_g, normed[:, c, :], gwT[:, c, :],
                                 start=(c == 0), stop=(c == NCH - 1))
            g_sb = small.tile([T, E], f32, tag="g")
            nc.vector.tensor_add(g_sb, ps_g, gb_b)
            m8 = small.tile([T, 8], f32, tag="m8")
            nc.vector.max(m8, g_sb)
            negm = small.tile([T, 1], f32, tag="negm")
            nc.scalar.mul(negm, m8[:, 0:1], -1.0)
            s4 = small.tile([T, K], f32, tag="s4")
            nc.scalar.activation(s4, m8[:, 0:K], AF.Exp, bias=negm, scale=1.0)
            den = small.tile([T, 1], f32, tag="den")
            nc.vector.reduce_sum(den, s4, axis=mybir.AxisListType.X)
            rden = small.tile([T, 1], f32, tag="rden")
            nc.vector.reciprocal(rden, den)
            ew = small.tile([T, K], f32, tag="ew")
            nc.vector.tensor_scalar_mul(ew, s4, rden)

            A = small.tile([T, E], f32, tag="A")
            for k in range(K):
                msk = small.tile([T, E], f32, tag="msk")
                nc.vector.tensor_scalar(msk, g_sb, m8[:, k:k + 1], None,
                                        op0=OP.is_equal)
                wm = small.tile([T, E], f32, tag="wm")
                nc.vector.tensor_scalar_mul(wm, msk, ew[:, k:k + 1])
                if k == 0:
                    nc.vector.tensor_copy(A, wm)
                else:
                    nc.vector.tensor_add(A, A, wm)
            # h_act is computed as silu(beta*glu)*(lin+1) = beta * true value;
            # fold the 1/beta into the per-expert routing scale.
            A_div = small.tile([T, E], f32, tag="A_div")
            nc.vector.tensor_scalar_mul(A_div, A, 1.0 / BETA)
            ps_a = ps_misc.tile([K, T], f32, tag="misc")
            nc.tensor.transpose(ps_a, A[:, 0:K], id64)
            a4t = small.tile([K, T], f32, tag="a4t")
            nc.scalar.copy(a4t, ps_a)

            # ---- bias-2 base: acc = A[:, :4] @ b2_shard ----
            acc = consts.tile([T, D], f32)
            ps_b1 = ps_o.tile([T, 512], f32, tag="o1", bufs=2)
            nc.tensor.matmul(ps_b1, a4t, b2_t[:, 0:512], start=True, stop=True)
            nc.scalar.copy(acc[:, 0:512], ps_b1)
            ps_b2 = ps_o.tile([T, 128], f32, tag="o2")
            nc.tensor.matmul(ps_b2, a4t, b2_t[:, 512:640], start=True, stop=True)
            nc.scalar.copy(acc[:, 512:640], ps_b2)

            # ---- experts: dense masked GLU FFN ----
            for e in range(EPC):
                w1_t, w2_t = w1_tiles[e], w2_tiles[e]
                ps_o1 = ps_o.tile([T, 512], f32, tag="o1", bufs=2)
                ps_o2 = ps_o.tile([T, 128], f32, tag="o2")
                for c in range(NCH):
                    ps_glu = ps_h.tile([128, T], f32, tag="h")
                    ps_lin = ps_h.tile([128, T], f32, tag="h")
                    for d in range(NCH):
                        nc.tensor.matmul(
                            ps_glu, w1_t[:, d, 128 * c:128 * (c + 1)],
                            normed[:, d, :],
                            start=(d == 0), stop=(d == NCH - 1))
                    for d in range(NCH):
                        nc.tensor.matmul(
                            ps_lin, w1_t[:, d, I + 128 * c:I + 128 * (c + 1)],
                            normed[:, d, :],
                            start=(d == 0), stop=(d == NCH - 1))
                    gm = hpool.tile([128, T], f32, tag="gm")
                    nc.vector.tensor_scalar(gm, ps_glu, b1T[:, c, e:e + 1],
                                            LIMIT, op0=OP.add, op1=OP.min)
                    p_ = hpool.tile([128, T], f32, tag="p")
                    nc.scalar.activation(p_, gm, AF.Silu, scale=BETA)
                    l1 = hpool.tile([128, T], f32, tag="l")
                    nc.vector.tensor_scalar(l1, ps_lin,
                                            b1T[:, NCH + c, e:e + 1],
                                            LIMIT, op0=OP.add, op1=OP.min)
                    nc.vector.tensor_scalar(l1, l1, -LIMIT, 1.0,
                                            op0=OP.max, op1=OP.add)
                    hact = hpool.tile([128, T], f32, tag="hact")
                    nc.vector.tensor_mul(hact, p_, l1)
                    nc.tensor.matmul(ps_o1, hact, w2_t[:, c, 0:512],
                                     start=(c == 0), stop=(c == NCH - 1))
                    nc.tensor.matmul(ps_o2, hact, w2_t[:, c, 512:640],
                                     start=(c == 0), stop=(c == NCH - 1))
                sc1 = small.tile([T, 512], f32, tag="sc1")
                nc.vector.tensor_scalar_mul(sc1, ps_o1, A_div[:, e:e + 1])
                nc.vector.tensor_add(acc[:, 0:512], acc[:, 0:512], sc1)
                sc2 = small.tile([T, 128], f32, tag="sc2")
                nc.vector.tensor_scalar_mul(sc2, ps_o2, A_div[:, e:e + 1])
                nc.vector.tensor_add(acc[:, 512:640], acc[:, 512:640], sc2)

            nc.sync.dma_start(out=dout.ap(), in_=acc)

    nc.finalize()
    return nc


def _get_nc():
    global _NC
    if _NC is None:
        _ensure_ntff_hook()
        _NC = _build()
    return _NC


def kernel(**inputs):
    global LAST_EXEC_NS
    nc = _get_nc()
    from concourse.bass_utils import run_bass_kernel_spmd

    x = np.ascontiguousarray(np.asarray(inputs["x"], dtype=np.float32))
    norm_w = np.ascontiguousarray(np.asarray(inputs["norm_w"], np.float32))
    gate_w = np.ascontiguousarray(np.asarray(inputs["gate_w"], np.float32))
    gate_b = np.ascontiguousarray(np.asarray(inputs["gate_b"], np.float32))
    w1 = np.asarray(inputs["w1"], np.float32)
    b1 = np.asarray(inputs["b1"], np.float32)
    w2 = np.asarray(inputs["w2"], np.float32)
    b2 = np.asarray(inputs["b2"], np.float32)

    x2 = np.ascontiguousarray(x[0, :, 0, :])  # (D, T)
    in_maps = []
    for c in range(NCORES):
        lo, hi = EPC * c, EPC * (c + 1)
        perm = np.r_[lo:hi, 0:lo, hi:E]
        in_maps.append({
            "x": x2,
            "norm_w": norm_w,
            "gate_w": np.ascontiguousarray(gate_w[perm]),
            "gate_b": np.ascontiguousarray(gate_b[perm]),
            "w1": np.ascontiguousarray(w1[lo:hi]),
            "b1": np.ascontiguousarray(b1[lo:hi]),
            "w2": np.ascontiguousarray(w2[lo:hi]),
            "b2": np.ascontiguousarray(b2[lo:hi]),
        })

    res = run_bass_kernel_spmd(nc, in_maps, core_ids=list(range(NCORES)),
                               trace=TRACE, tmpdir=PROF_DIR)
    LAST_EXEC_NS = res.exec_time_ns
    total = np.sum([r["out"] for r in res.results], axis=0)  # (T, D)
    return (x + total.T[None, :, None, :]).astype(np.float32)


# revision 9
# speedup vs baseline: 2.0080x; 1.9985x over previous
"""MoE block (RMSNorm + top-4 router + 32-expert GLU FFN) on 8 TRN2 NeuronCores.

Expert-parallel: core c owns experts [4c, 4c+4). Each core redundantly
computes the (tiny) RMSNorm + router over all 32 experts in f32, then runs a
dense masked FFN over all 64 tokens for its own 4 experts in bf16 (weights
host-cast; PSUM accumulation is f32), scaling each expert's contribution by
the routing weight (0 for unrouted tokens). gate_w/gate_b are passed to each
core with its own 4 experts permuted to rows 0..3, so the SPMD program
always reads routing columns 0..3 — no core-id branching.

The host sums the 8 partial (T, D) outputs and adds the residual — that is
the "unshard" for expert parallelism.
"""

import sys
import types

sys.path.insert(0, "/opt/trn_rl_repo")

import numpy as np

D = 640
I = 640
E = 32
T = 64
K = 4
EPS = 1e-5
LIMIT = 7.0
BETA = 1.702
NCORES = 8
EPC = E // NCORES          # experts per core
NCH = D // 128             # 5 contraction chunks of 128
NIH = 2 * I // 128         # 10 partition chunks of h

TRACE = False
PROF_DIR = None
LAST_EXEC_NS = None

_NC = None


def _ensure_ntff_hook():
    """boot() skips NTFF hook registration (image antenv lacks axon_hooks);
    provide the module so bass_utils can profile when TRACE=True."""
    if "antenv.axon_hooks" in sys.modules:
        return
    try:
        from trn_agent_boot.trn_boot import _ntff_profile_via_ctypes
        hook = _ntff_profile_via_ctypes("/opt/axon/libaxon_pjrt.so")
    except Exception:
        hook = None
    m = types.ModuleType("antenv.axon_hooks")
    m.get_axon_ntff_profile_hook = lambda: hook
    m.set_axon_ntff_profile_hook = lambda h: None
    sys.modules["antenv.axon_hooks"] = m


def _build():
    import concourse.bass as bass
    import concourse.bacc as bacc
    import concourse.tile as tile
    from concourse import mybir
    from concourse.masks import make_identity

    f32 = mybir.dt.float32
    bf16 = mybir.dt.bfloat16
    AF = mybir.ActivationFunctionType
    OP = mybir.AluOpType

    nc = bacc.Bacc("TRN2", target_bir_lowering=False, debug=False,
                   num_devices=NCORES)
    dx = nc.dram_tensor("x", (D, T), f32, kind="ExternalInput")
    dnw = nc.dram_tensor("norm_w", (D,), f32, kind="ExternalInput")
    dgw = nc.dram_tensor("gate_w", (E, D), f32, kind="ExternalInput")
    dgb = nc.dram_tensor("gate_b", (E,), f32, kind="ExternalInput")
    dw1 = nc.dram_tensor("w1", (EPC, D, 2 * I), bf16, kind="ExternalInput")
    db1 = nc.dram_tensor("b1", (EPC, 2 * I), bf16, kind="ExternalInput")
    dw2 = nc.dram_tensor("w2", (EPC, I, D), bf16, kind="ExternalInput")
    db2 = nc.dram_tensor("b2", (EPC, D), f32, kind="ExternalInput")
    dout = nc.dram_tensor("out", (T, D), f32, kind="ExternalOutput")

    with tile.TileContext(nc) as tc:
        with (
            tc.tile_pool(name="consts", bufs=1) as consts,
            tc.tile_pool(name="small", bufs=2) as small,
            tc.tile_pool(name="wpool", bufs=3) as wpool,
            tc.tile_pool(name="hpool", bufs=3) as hpool,
            tc.tile_pool(name="ps_misc", bufs=1, space="PSUM") as ps_misc,
            tc.tile_pool(name="ps_h", bufs=4, space="PSUM") as ps_h,
            tc.tile_pool(name="ps_o", bufs=1, space="PSUM") as ps_o,
        ):
            # ---- small input loads (ACT HWDGE ring; SP ring carries the
            # big expert-weight streams) ----
            x_t = consts.tile([128, NCH, T], f32)
            nc.scalar.dma_start(out=x_t,
                                in_=dx.ap().rearrange("(c p) t -> p c t", p=128))
            nw_t = consts.tile([128, NCH], f32)
            nc.scalar.dma_start(out=nw_t,
                                in_=dnw.ap().rearrange("(c p) -> p c", p=128))
            gwn = consts.tile([E, D], f32)
            nc.scalar.dma_start(out=gwn, in_=dgw.ap())
            gb_b = consts.tile([T, E], f32)
            gb_base = dgb.ap()
            nc.scalar.dma_start(
                out=gb_b,
                in_=bass.AP(tensor=gb_base.tensor, offset=0,
                            ap=[[0, T], [1, E]]))
            b1_sb = consts.tile([EPC, 2 * I], bf16)
            nc.scalar.dma_start(out=b1_sb, in_=db1.ap())
            b2_t = consts.tile([EPC, D], f32)
            nc.scalar.dma_start(out=b2_t, in_=db2.ap())

            ones128 = consts.tile([128, 128], f32)
            nc.vector.memset(ones128, 1.0)
            ones_bf = consts.tile([1, T], bf16)
            nc.vector.memset(ones_bf, 1.0)
            eps_t = consts.tile([128, 1], f32)
            nc.vector.memset(eps_t, EPS)
            id128 = consts.tile([128, 128], f32)
            make_identity(nc, id128)

            # ---- expert weight streams (big; SP HWDGE ring) ----
            w1r = dw1.ap().rearrange("e (c p) i -> e p c i", p=128)
            w2r = dw2.ap().rearrange("e (c p) i -> e p c i", p=128)
            w1_tiles, w2_tiles = [], []
            for e in range(EPC):
                w1_t = wpool.tile([128, NCH, 2 * I], bf16, tag="w1")
                nc.sync.dma_start(out=w1_t, in_=w1r[e])
                w2_t = wpool.tile([128, NCH, D], bf16, tag="w2")
                nc.sync.dma_start(out=w2_t, in_=w2r[e])
                w1_tiles.append(w1_t)
                w2_tiles.append(w2_t)

            # gate_w.T (D on partitions) via PE transpose of the native load
            gwT = consts.tile([128, NCH, E], f32)
            for c in range(NCH):
                ps_t = ps_misc.tile([128, E], f32, tag="misc")
                nc.tensor.transpose(ps_t, gwn[:, 128 * c:128 * (c + 1)],
                                    id128[0:E, 0:E])
                nc.scalar.copy(gwT[:, c, :], ps_t)

            # ---- RMSNorm (x is (D, T); D on partitions) ----
            xx = small.tile([128, NCH, T], f32, tag="xx")
            nc.vector.tensor_mul(xx, x_t, x_t)
            ps_ss = ps_misc.tile([128, T], f32, tag="misc")
            for c in range(NCH):
                # ones.T @ xx chunk: broadcasts sum over D to all partitions
                nc.tensor.matmul(ps_ss, ones128, xx[:, c, :],
                                 start=(c == 0), stop=(c == NCH - 1))
            sq = small.tile([128, T], f32, tag="sq")
            nc.scalar.activation(sq, ps_ss, AF.Sqrt, bias=eps_t, scale=1.0 / D)
            rstd = small.tile([128, T], f32, tag="rstd")
            nc.vector.reciprocal(rstd, sq)
            normed = consts.tile([128, NCH, T], f32)
            for c in range(NCH):
                xn = small.tile([128, T], f32, tag="xn")
                nc.vector.tensor_scalar_mul(xn, x_t[:, c, :], nw_t[:, c:c + 1])
                nc.vector.tensor_mul(normed[:, c, :], xn, rstd)
            normed_bf = consts.tile([128, NCH, T], bf16)
            nc.vector.tensor_copy(normed_bf, normed)

            # ---- router: gate, top-4, softmax, dense routing matrix A ----
            ps_g = ps_misc.tile([T, E], f32, tag="misc")
            for c in range(NCH):
                nc.tensor.matmul(ps_g, normed[:, c, :], gwT[:, c, :],
                                 start=(c == 0), stop=(c == NCH - 1))
            g_sb = small.tile([T, E], f32, tag="g")
            nc.vector.tensor_add(g_sb, ps_g, gb_b)
            m8 = small.tile([T, 8], f32, tag="m8")
            nc.vector.max(m8, g_sb)
            negm = small.tile([T, 1], f32, tag="negm")
            nc.scalar.mul(negm, m8[:, 0:1], -1.0)
            s4 = small.tile([T, K], f32, tag="s4")
            nc.scalar.activation(s4, m8[:, 0:K], AF.Exp, bias=negm, scale=1.0)
            den = small.tile([T, 1], f32, tag="den")
            nc.vector.reduce_sum(den, s4, axis=mybir.AxisListType.X)
            rden = small.tile([T, 1], f32, tag="rden")
            nc.vector.reciprocal(rden, den)
            ew = small.tile([T, K], f32, tag="ew")
            nc.vector.tensor_scalar_mul(ew, s4, rden)

            A = small.tile([T, E], f32, tag="A")
            for k in range(K):
                msk = small.tile([T, E], f32, tag="msk")
                nc.vector.tensor_scalar(msk, g_sb, m8[:, k:k + 1], None,
                                        op0=OP.is_equal)
                wm = small.tile([T, E], f32, tag="wm")
                nc.vector.tensor_scalar_mul(wm, msk, ew[:, k:k + 1])
                if k == 0:
                    nc.vector.tensor_copy(A, wm)
                else:
                    nc.vector.tensor_add(A, A, wm)
            # h_act is computed as silu(beta*glu)*(lin+1) = beta * true value;
            # fold the 1/beta into the per-expert routing scale.
            A_div = small.tile([T, E], f32, tag="A_div")
            nc.vector.tensor_scalar_mul(A_div, A, 1.0 / BETA)
            ps_a = ps_misc.tile([K, T], f32, tag="misc")
            nc.tensor.transpose(ps_a, A[:, 0:K], id128[0:T, 0:T])
            a4t = small.tile([K, T], f32, tag="a4t")
            nc.scalar.copy(a4t, ps_a)

            # ---- bias-2 base: acc = A[:, :4] @ b2_shard ----
            acc = consts.tile([T, D], f32)
            ps_b1 = ps_o.tile([T, 512], f32, tag="o1", bufs=2)
            nc.tensor.matmul(ps_b1, a4t, b2_t[:, 0:512], start=True, stop=True)
            nc.scalar.copy(acc[:, 0:512], ps_b1)
            ps_b2 = ps_o.tile([T, 128], f32, tag="o2")
            nc.tensor.matmul(ps_b2, a4t, b2_t[:, 512:640], start=True, stop=True)
            nc.scalar.copy(acc[:, 512:640], ps_b2)

            # ---- experts: dense masked GLU FFN (bf16 matmuls, f32 PSUM) ----
            for e in range(EPC):
                w1_t, w2_t = w1_tiles[e], w2_tiles[e]
                ps_o1 = ps_o.tile([T, 512], f32, tag="o1", bufs=2)
                ps_o2 = ps_o.tile([T, 128], f32, tag="o2")
                for c in range(NCH):
                    ps_glu = ps_h.tile([128, T], f32, tag="h")
                    ps_lin = ps_h.tile([128, T], f32, tag="h")
                    # bias via rank-1 matmul: b1_chunk.T @ ones == b1 ⊗ 1
                    nc.tensor.matmul(ps_glu,
                                     b1_sb[0:1, 128 * c:128 * (c + 1)],
                                     ones_bf, start=True, stop=False)
                    for d in range(NCH):
                        nc.tensor.matmul(
                            ps_glu, w1_t[:, d, 128 * c:128 * (c + 1)],
                            normed_bf[:, d, :],
                            start=False, stop=(d == NCH - 1))
                    nc.tensor.matmul(ps_lin,
                                     b1_sb[0:1, I + 128 * c:I + 128 * (c + 1)],
                                     ones_bf, start=True, stop=False)
                    for d in range(NCH):
                        nc.tensor.matmul(
                            ps_lin, w1_t[:, d, I + 128 * c:I + 128 * (c + 1)],
                            normed_bf[:, d, :],
                            start=False, stop=(d == NCH - 1))
                    gm = hpool.tile([128, T], f32, tag="gm")
                    nc.vector.tensor_scalar(gm, ps_glu, LIMIT, None,
                                            op0=OP.min)
                    p_ = hpool.tile([128, T], f32, tag="p")
                    nc.scalar.activation(p_, gm, AF.Silu, scale=BETA)
                    l1 = hpool.tile([128, T], f32, tag="l")
                    nc.vector.tensor_scalar(l1, ps_lin, LIMIT, -LIMIT,
                                            op0=OP.min, op1=OP.max)
                    l2 = hpool.tile([128, T], f32, tag="l2")
                    nc.scalar.add(l2, l1, 1.0)
                    hact = hpool.tile([128, T], bf16, tag="hact")
                    nc.vector.tensor_mul(hact, p_, l2)
                    nc.tensor.matmul(ps_o1, hact, w2_t[:, c, 0:512],
                                     start=(c == 0), stop=(c == NCH - 1))
                    nc.tensor.matmul(ps_o2, hact, w2_t[:, c, 512:640],
                                     start=(c == 0), stop=(c == NCH - 1))
                sc1 = small.tile([T, 512], f32, tag="sc1")
                nc.scalar.activation(sc1, ps_o1, AF.Copy,
                                     scale=A_div[:, e:e + 1])
                nc.vector.tensor_add(acc[:, 0:512], acc[:, 0:512], sc1)
                sc2 = small.tile([T, 128], f32, tag="sc2")
                nc.scalar.activation(sc2, ps_o2, AF.Copy,
                                     scale=A_div[:, e:e + 1])
                nc.vector.tensor_add(acc[:, 512:640], acc[:, 512:640], sc2)

            nc.scalar.dma_start(out=dout.ap(), in_=acc)

    nc.finalize()
    return nc


def _get_nc():
    global _NC
    if _NC is None:
        _ensure_ntff_hook()
        _NC = _build()
    return _NC


def kernel(**inputs):
    global LAST_EXEC_NS
    nc = _get_nc()
    import ml_dtypes
    from concourse.bass_utils import run_bass_kernel_spmd

    bf = ml_dtypes.bfloat16
    x = np.ascontiguousarray(np.asarray(inputs["x"], dtype=np.float32))
    norm_w = np.ascontiguousarray(np.asarray(inputs["norm_w"], np.float32))
    gate_w = np.ascontiguousarray(np.asarray(inputs["gate_w"], np.float32))
    gate_b = np.ascontiguousarray(np.asarray(inputs["gate_b"], np.float32))
    w1 = np.asarray(inputs["w1"], np.float32).astype(bf)
    b1 = np.asarray(inputs["b1"], np.float32).astype(bf)
    w2 = np.asarray(inputs["w2"], np.float32).astype(bf)
    b2 = np.asarray(inputs["b2"], np.float32)

    x2 = np.ascontiguousarray(x[0, :, 0, :])  # (D, T)
    in_maps = []
    for c in range(NCORES):
        lo, hi = EPC * c, EPC * (c + 1)
        perm = np.r_[lo:hi, 0:lo, hi:E]
        in_maps.append({
            "x": x2,
            "norm_w": norm_w,
            "gate_w": np.ascontiguousarray(gate_w[perm]),
            "gate_b": np.ascontiguousarray(gate_b[perm]),
            "w1": np.ascontiguousarray(w1[lo:hi]),
            "b1": np.ascontiguousarray(b1[lo:hi]),
            "w2": np.ascontiguousarray(w2[lo:hi]),
            "b2": np.ascontiguousarray(b2[lo:hi]),
        })

    res = run_bass_kernel_spmd(nc, in_maps, core_ids=list(range(NCORES)),
                               trace=TRACE, tmpdir=PROF_DIR)
    LAST_EXEC_NS = res.exec_time_ns
    total = np.sum([r["out"] for r in res.results], axis=0)  # (T, D)
    return (x + total.T[None, :, None, :]).astype(np.float32)
